# revision 5
# baseline (speedup 1.0000x reference)
"""Cross-attention (b=1, n=2048, dim=1024, 16 heads x 64) on 8 TRN2 NeuronCores.

Strategy:
- Tensor-parallel over heads: core k computes heads (2k, 2k+1) end to end and a
  partial output projection; host sums the 8 partials (the Wo all-reduce).
- Mask compaction on host: the padded mask pm gates both rows and columns of
  the attention matrix. Masked ROWS get uniform attention = (mean v) @ Wo,
  computed exactly on host; masked COLUMNS contribute exp(-inf)=0. So the
  device only computes attention over the C0 unmasked positions (padded to a
  multiple of 128), roughly halving all n^2 work.
- float32r matmuls everywhere (1 cycle/row at N>=256, ~2^-11 rounding, far
  more accurate than bf16). Softmax without max-subtraction (scores are O(1)
  by construction), with column-pad masking folded into the Exp activation's
  per-partition bias.
- Activations enter transposed (contraction on partitions) via bf16 hi/lo
  DMA-transpose + on-chip recombine to f32r (fp32 has no DMA-transpose path).
"""
import numpy as np

N_CORES = 8
HEADS = 16
DH = 64  # head dim
DIM = 1024
HPC = HEADS // N_CORES  # heads per core = 2

_cache = {}


def _build(C, JB, chunks):
    """Build + schedule the per-core Bass program for padded length C."""
    import concourse.bass as bass
    import concourse.mybir as mybir
    import concourse.tile as tile
    from concourse import bacc
    from concourse.masks import make_identity

    F32 = mybir.dt.float32
    F32R = mybir.dt.float32r
    BF16 = mybir.dt.bfloat16
    EXP = mybir.ActivationFunctionType.Exp
    scale = DIM ** -0.5
    CB = DIM // 128  # contraction blocks for projections (8)

    nc = bacc.Bacc("TRN2", target_bir_lowering=False, debug=False)

    xhi_d = nc.dram_tensor("xhi", [C, DIM], BF16, kind="ExternalInput").ap()
    xlo_d = nc.dram_tensor("xlo", [C, DIM], BF16, kind="ExternalInput").ap()
    mhi_d = nc.dram_tensor("mhi", [C, DIM], BF16, kind="ExternalInput").ap()
    mlo_d = nc.dram_tensor("mlo", [C, DIM], BF16, kind="ExternalInput").ap()
    wq_d = nc.dram_tensor("wq", [DIM, 128], F32, kind="ExternalInput").ap()
    wk_d = nc.dram_tensor("wk", [DIM, 128], F32, kind="ExternalInput").ap()
    wv_d = nc.dram_tensor("wv", [DIM, 128], F32, kind="ExternalInput").ap()
    wo_d = nc.dram_tensor("wo", [128, DIM], F32, kind="ExternalInput").ap()
    jb_d = nc.dram_tensor("jbias", [128, JB], F32, kind="ExternalInput").ap()
    out_d = nc.dram_tensor("out", [C, DIM], F32, kind="ExternalOutput").ap()

    with tile.TileContext(nc) as tc:
        with (
            tc.tile_pool(name="persist", bufs=1) as pp,
            tc.tile_pool(name="work", bufs=3) as wk,
            tc.tile_pool(name="outstage", bufs=3) as outp,
        ):
            # ---- persistent tiles ----
            qT = pp.tile([128, C], F32R)  # [d(2 heads), i]
            kT = pp.tile([128, C], F32R)
            vT = pp.tile([128, C], F32R)
            v1 = pp.tile([128, JB, HPC, DH], F32R)  # v natural per (jblock, head)
            onesw = pp.tile([128, DH], F32R)  # all-ones lhsT for den matmuls
            wo_r = pp.tile([DH, HPC, DIM], F32R)  # Wo rows per head, partitions 0-63
            w_r = {}
            for nm in ("wq", "wk", "wv"):
                w_r[nm] = pp.tile([128, CB, 128], F32R, name=f"w_{nm}", tag=f"w_{nm}")
            jbias = pp.tile([128, JB], F32)
            ident = pp.tile([128, DH], F32R)
            ON = [pp.tile([DH, C], F32R, name=f"ON{h}", tag=f"ON{h}") for h in range(HPC)]

            nc.gpsimd.dma_start(jbias[:], jb_d)

            # stacked double-identity (I_64 in each partition half) so the
            # PE transpose rhs can match lhsT base partition 0 or 64
            id32 = wk.tile([128, DH], F32, tag="id32")
            make_identity(nc, id32[0:DH, :])
            make_identity(nc, id32[DH:128, :])
            nc.vector.tensor_copy(ident[:], id32[:])

            ones32 = wk.tile([128, DH], F32, tag="ones32")
            nc.vector.memset(ones32[:], 1.0)
            nc.vector.tensor_copy(onesw[:], ones32[:])

            # ---- weights: load fp32, cast to f32r ----
            for nm, d_ap in (("wq", wq_d), ("wk", wk_d), ("wv", wv_d)):
                wtmp = wk.tile([128, CB, 128], F32, tag="wtmp")
                nc.gpsimd.dma_start(
                    wtmp[:], d_ap.rearrange("(cb p) d -> p cb d", p=128)
                )
                nc.vector.tensor_copy(w_r[nm][:], wtmp[:])
            wotmp = wk.tile([DH, HPC, DIM], F32, tag="wotmp")
            nc.gpsimd.dma_start(
                wotmp[:], wo_d.rearrange("(h p) e -> p h e", p=DH)
            )
            nc.vector.tensor_copy(wo_r[:], wotmp[:])

            with (
                tc.tile_pool(name="actsT", bufs=1) as ap_,
                tc.tile_pool(name="hilo", bufs=4) as hl,
                tc.tile_pool(name="psB", bufs=2, space="PSUM") as psB,
                tc.tile_pool(name="psT", bufs=2, space="PSUM") as psT,
            ):
                # ---- phase A: transposed activations (bf16 hi/lo -> f32r) ----
                xT = ap_.tile([128, CB, C], F32R)
                mT = ap_.tile([128, CB, C], F32R)
                for src_hi, src_lo, dst in ((xhi_d, xlo_d, xT), (mhi_d, mlo_d, mT)):
                    for cb in range(CB):
                        thi = hl.tile([128, C], BF16, tag="thi")
                        tlo = hl.tile([128, C], BF16, tag="tlo")
                        nc.sync.dma_start_transpose(
                            thi[:], src_hi[:, cb * 128 : (cb + 1) * 128]
                        )
                        nc.sync.dma_start_transpose(
                            tlo[:], src_lo[:, cb * 128 : (cb + 1) * 128]
                        )
                        with nc.allow_low_precision(reason="bf16 hi/lo recombine"):
                            nc.vector.tensor_add(dst[:, cb, :], thi[:], tlo[:])

                # ---- phase B: projections qT/kT/vT = W.T @ actT ----
                for i0, cw in chunks:
                    for nm, src, dst in (
                        ("wq", xT, qT),
                        ("wk", mT, kT),
                        ("wv", mT, vT),
                    ):
                        ps = psB.tile([128, 512], F32, tag="projps")
                        for cb in range(CB):
                            nc.tensor.matmul(
                                ps[:, :cw],
                                w_r[nm][:, cb, :],
                                src[:, cb, i0 : i0 + cw],
                                start=(cb == 0),
                                stop=(cb == CB - 1),
                            )
                        with nc.allow_low_precision(reason="psum->f32r"):
                            nc.vector.tensor_copy(dst[:, i0 : i0 + cw], ps[:, :cw])

                # ---- phase B2: v natural via PE transpose ----
                for h in range(HPC):
                    for jb in range(JB):
                        tp = psT.tile([128, DH], F32R, tag="vtp")
                        nc.tensor.transpose(
                            tp[:],
                            vT[h * DH : (h + 1) * DH, jb * 128 : (jb + 1) * 128],
                            ident[h * DH : (h + 1) * DH, :],
                        )
                        with nc.allow_low_precision(reason="psum->f32r"):
                            nc.scalar.copy(v1[:, jb, h, :], tp[:])

            # ---- phase C: attention ----
            with (
                tc.tile_pool(name="ptp", bufs=2) as ptp,
                tc.tile_pool(name="nrm", bufs=3) as nrm,
                tc.tile_pool(name="psS", bufs=2, space="PSUM") as psS,
                tc.tile_pool(name="psO", bufs=2, space="PSUM") as psO,
                tc.tile_pool(name="psD", bufs=2, space="PSUM") as psD,
            ):
                for i0, cw in chunks:
                    PT = ptp.tile([128, JB, HPC, 512], F32R, tag="PT")
                    for jb in range(JB):
                        sps = psS.tile([128, HPC, 512], F32, tag="S")
                        for h in range(HPC):
                            nc.tensor.matmul(
                                sps[:, h, :cw],
                                kT[h * DH : (h + 1) * DH, jb * 128 : (jb + 1) * 128],
                                qT[h * DH : (h + 1) * DH, i0 : i0 + cw],
                                start=True,
                                stop=True,
                            )
                        with nc.allow_low_precision(reason="exp->f32r"):
                            nc.scalar.activation(
                                PT[:, jb, :, :cw],
                                sps[:, :, :cw],
                                EXP,
                                bias=jbias[:, jb : jb + 1],
                                scale=scale,
                            )
                    for h in range(HPC):
                        ops = psO.tile([DH, 512], F32, tag="O")
                        dps = psD.tile([DH, 512], F32, tag="den")
                        for jb in range(JB):
                            nc.tensor.matmul(
                                ops[:, :cw],
                                v1[:, jb, h, :],
                                PT[:, jb, h, :cw],
                                start=(jb == 0),
                                stop=(jb == JB - 1),
                            )
                            nc.tensor.matmul(
                                dps[:, :cw],
                                onesw[:],
                                PT[:, jb, h, :cw],
                                start=(jb == 0),
                                stop=(jb == JB - 1),
                            )
                        recd = nrm.tile([DH, 512], F32R, tag="recd")
                        with nc.allow_low_precision(reason="recip f32r"):
                            nc.vector.reciprocal(recd[:, :cw], dps[:, :cw])
                        osb = nrm.tile([DH, 512], F32, tag="osb")
                        nc.scalar.copy(osb[:, :cw], ops[:, :cw])
                        with nc.allow_low_precision(reason="normalize f32r"):
                            nc.vector.tensor_mul(
                                ON[h][:, i0 : i0 + cw], osb[:, :cw], recd[:, :cw]
                            )

            # ---- phase D: partial output projection ----
            with tc.tile_pool(name="psE", bufs=4, space="PSUM") as psE:
                for isub in range(C // 128):
                    ob = outp.tile([128, DIM], F32, tag="ob")
                    for eb in range(DIM // 512):
                        dp = psE.tile([128, 512], F32, tag="dout")
                        for h in range(HPC):
                            nc.tensor.matmul(
                                dp[:],
                                ON[h][:, isub * 128 : (isub + 1) * 128],
                                wo_r[:, h, eb * 512 : (eb + 1) * 512],
                                start=(h == 0),
                                stop=(h == HPC - 1),
                            )
                        if eb % 2 == 0:
                            nc.vector.tensor_copy(ob[:, eb * 512 : (eb + 1) * 512], dp[:])
                        else:
                            nc.scalar.copy(ob[:, eb * 512 : (eb + 1) * 512], dp[:])
                    nc.sync.dma_start(out_d[isub * 128 : (isub + 1) * 128, :], ob[:])

    nc.compile()
    return nc


def _get_program(C, JB, chunks):
    key = (C, JB, tuple(chunks))
    if key not in _cache:
        _cache[key] = _build(C, JB, chunks)
    return _cache[key]


def kernel(x, m, mask, Wq, Wk, Wv, Wo, bo, _trace=False, _bass_results=None):
    import ml_dtypes
    from concourse.bass_utils import run_bass_kernel_spmd

    bf16 = ml_dtypes.bfloat16
    x = np.asarray(x)
    m = np.asarray(m)
    mask = np.asarray(mask)
    Wq, Wk, Wv, Wo, bo = (np.asarray(a, np.float32) for a in (Wq, Wk, Wv, Wo, bo))
    b, n, dim = x.shape
    assert (b, dim) == (1, DIM)

    pm = np.concatenate([np.array([True]), mask[0]])  # [n]
    sel = np.nonzero(pm)[0]
    C0 = len(sel)
    C = max(((C0 + 127) // 128) * 128, 256)
    JB = C // 128
    chunks = []
    i0 = 0
    while i0 < C:
        cw = min(512, C - i0)
        chunks.append((i0, cw))
        i0 += cw

    x_c = np.zeros((C, DIM), np.float32)
    x_c[:C0] = x[0][sel]
    m_c = np.zeros((C, DIM), np.float32)
    m_c[:C0] = m[0][sel]

    def hilo(a):
        hi = a.astype(bf16)
        lo = (a - hi.astype(np.float32)).astype(bf16)
        return hi, lo

    xhi, xlo = hilo(x_c)
    mhi, mlo = hilo(m_c)

    jbias = np.zeros((JB, 128), np.float32)
    flat = np.zeros(C, np.float32)
    flat[C0:] = -1e30
    jbias[:] = flat.reshape(JB, 128)
    jbias_t = np.ascontiguousarray(jbias.T)  # [128, JB]

    nc = _get_program(C, JB, chunks)

    in_maps = []
    for c in range(N_CORES):
        h0 = c * HPC * DH  # 128*c
        in_maps.append(
            {
                "xhi": xhi,
                "xlo": xlo,
                "mhi": mhi,
                "mlo": mlo,
                "wq": np.ascontiguousarray(Wq[:, h0 : h0 + HPC * DH]),
                "wk": np.ascontiguousarray(Wk[:, h0 : h0 + HPC * DH]),
                "wv": np.ascontiguousarray(Wv[:, h0 : h0 + HPC * DH]),
                "wo": np.ascontiguousarray(Wo[h0 : h0 + HPC * DH, :]),
                "jbias": jbias_t,
            }
        )

    res = run_bass_kernel_spmd(
        nc, in_maps, core_ids=list(range(N_CORES)), trace=_trace
    )
    if _bass_results is not None:
        _bass_results.append(res)

    acc = np.sum(
        np.stack([r["out"][:C0] for r in res.results]), axis=0, dtype=np.float64
    )

    # host-side: masked rows get uniform attention over ALL positions
    mv = m[0].astype(np.float64).mean(axis=0)  # mean over all j of m
    mv_out = (mv @ Wv.astype(np.float64)) @ Wo.astype(np.float64)  # [dim]

    out = np.empty((n, DIM), np.float64)
    out[sel] = acc
    out[~pm] = mv_out
    out += bo.astype(np.float64)
    return out[None].astype(np.float32)
